# revision 14
# baseline (speedup 1.0000x reference)
"""MoE top-1 routing + expert MLP + LayerNorm on 8 Trainium2 NeuronCores.

Expert-parallel: core e holds expert e's weights (E == n_cores == 8).
Host computes the (cheap) top-1 gate routing and does the token
dispatch/undispatch as part of sharding; each core runs its expert's MLP
(fp32r matmuls, exact gelu) and the final LayerNorm on its own tokens.

The top_val gate scaling of the reference is a mathematical no-op here:
LayerNorm is scale-invariant per token up to the eps placement
(top_val in [1-8e-8, 1)), so it is skipped (validated: rel err 6e-5).

Self-contained: hardcodes B=4, T=2048, D=1024, H=2048, E=8.
"""

import sys

sys.path.insert(0, "/opt/trn_rl_repo")

import numpy as np

import concourse.bass as bass
import concourse.bacc as bacc
import concourse.mybir as mybir
import concourse.tile as tile
from concourse.bass_utils import run_bass_kernel_spmd

D = 1024
H = 2048
E = 8
LN_EPS = 1e-5

F32 = mybir.dt.float32
F32R = mybir.dt.float32r
AF = mybir.ActivationFunctionType
OP = mybir.AluOpType

KD = D // 128   # 8  k-tiles of the d_model contraction
MH = H // 128   # 16 h-tiles of the hidden dim


def _chunk_sizes(C, cap):
    """Split C (multiple of 128, >=256) into chunks of 256..cap, mult of 128.

    cap=512 puts a big chunk first (more PE work to overlap the weight-DMA
    window at kernel start); cap=384 keeps pools small when SBUF is tight."""
    if cap >= 512:
        sizes = []
        rem = C
        while rem > 0:
            take = rem if rem <= 512 else (384 if rem == 640 else 512)
            sizes.append(take)
            rem -= take
    else:
        nch = -(-C // cap)
        base = (C // nch) // 128 * 128
        sizes = [base] * nch
        rem = (C - base * nch) // 128
        for i in range(rem):
            sizes[i] += 128
    assert sum(sizes) == C and all(
        c % 128 == 0 and 256 <= c <= 512 for c in sizes
    ), sizes
    return sizes


def build_program(C, with_affine, act=None, repeat=1):
    """Build the per-core Bass program: C tokens through one expert + LN."""
    if act is None:
        act = AF.Gelu
    # the gb tile (8 KB/partition) pushes the 512-chunk layout over SBUF
    chunks = _chunk_sizes(C, cap=384 if with_affine else 512)
    nc = bacc.Bacc("TRN2", target_bir_lowering=False, debug=False, num_devices=E)

    xt_d = nc.dram_tensor("xt", [128, KD, C], F32R, kind="ExternalInput")
    w1_d = nc.dram_tensor("w1", [KD, 128, H], F32R, kind="ExternalInput")
    w2_d = nc.dram_tensor("w2", [KD, 128, H], F32R, kind="ExternalInput")
    if with_affine:
        gb_d = nc.dram_tensor("gb", [128, 2 * D], F32, kind="ExternalInput")
    out_d = nc.dram_tensor("out", [C, D], F32, kind="ExternalOutput")

    with tile.TileContext(nc) as tc:
        with (
            tc.tile_pool(name="wts", bufs=1) as wts,
            tc.tile_pool(name="xp", bufs=2) as xp,
            tc.tile_pool(name="gp", bufs=MH) as gp,
            tc.tile_pool(name="sp", bufs=3) as sp,
            tc.tile_pool(name="st", bufs=2) as st,
            tc.tile_pool(name="ps", bufs=8, space=bass.MemorySpace.PSUM) as ps,
        ):
            import contextlib

            rep_ctx = (
                tc.For_i(
                    0, repeat, 1, name="rep",
                    hint_engines=(mybir.EngineType.PE,),
                )
                if repeat > 1
                else contextlib.nullcontext()
            )

            # ---- chunk-0 tokens first so stage 1 can start ASAP
            def load_xt(ci, c0, cs):
                t = xp.tile([128, KD, cs], F32R, tag="xt", name=f"xt_{ci}")
                nc.sync.dma_start(t[:], xt_d[:, :, c0 : c0 + cs])
                return t

            offs = np.cumsum([0] + chunks).tolist()

            # constants + HAM-warmup garbage tiles live outside the bench loop
            eps_t = wts.tile([128, 1], F32, tag="eps")
            nc.vector.memset(eps_t[:], LN_EPS)
            warm_sb = wts.tile([128, 512], F32R, tag="warm")
            nc.vector.memset(warm_sb[:].bitcast(F32), 0.0)

            rep_ctx.__enter__()

            # dummy matmuls: keep the PE busy (and the HAM clock-gate warm)
            # while the first weight/activation DMAs land
            warm_ps = ps.tile([128, 384], F32, tag="ps", name="warm_ps")
            for i in range(16):
                nc.tensor.matmul(
                    warm_ps[:],
                    warm_sb[:, 0:128],
                    warm_sb[:, 128:512],
                    start=True,
                    stop=True,
                )

            # chunk-0 tokens + w1, interleaved per k-tile so stage 1 can
            # start after ~1 MB instead of ~10 MB
            cs0 = chunks[0]
            xt_sb = xp.tile([128, KD, cs0], F32R, tag="xt", name="xt_0")
            w1_sb = []
            for k in range(KD):
                nc.sync.dma_start(xt_sb[:, k, :], xt_d[:, k, 0:cs0])
                t = wts.tile([128, H], F32R, tag=f"w1_{k}", name=f"w1sb_{k}")
                nc.sync.dma_start(t[:], w1_d[k])
                w1_sb.append(t)

            def stage1(xt_t, cs):
                """G^T = gelu(W1^T X^T) for one token chunk. Returns 16 [128,cs] tiles."""
                gt = [None] * MH
                for half in range(2):
                    pst = [ps.tile([128, cs], F32, tag="ps", name=f"ps1_{half}_{i}") for i in range(8)]
                    for k in range(KD):
                        rhs = xt_t[:, k, :]
                        for h8 in range(8):
                            h = half * 8 + h8
                            nc.tensor.matmul(
                                pst[h8][:],
                                w1_sb[k][:, h * 128 : (h + 1) * 128],
                                rhs,
                                start=(k == 0),
                                stop=(k == KD - 1),
                            )
                    for h8 in range(8):
                        h = half * 8 + h8
                        g = gp.tile([128, cs], F32R, tag="gt", name=f"gt_{h}")
                        nc.scalar.activation(g[:], pst[h8][:], act)
                        gt[h] = g
                return gt

            gt = stage1(xt_sb, cs0)

            # ---- second-phase weights (and LN affine), behind w1 in queue order
            w2_sb = []
            for j in range(KD):
                t = wts.tile([128, H], F32R, tag=f"w2_{j}", name=f"w2sb_{j}")
                nc.sync.dma_start(t[:], w2_d[j])
                w2_sb.append(t)
            if with_affine:
                gb_sb = wts.tile([128, 2 * D], F32, tag="gb")
                nc.sync.dma_start(gb_sb[:], gb_d[:])

            def w2_slice(m, n):
                # W2 m-tile [128, D] packed in pairs: tile j=m//2, cols (m%2)*D
                j, r = divmod(m, 2)
                return w2_sb[j][:, r * D + n * 512 : r * D + (n + 1) * 512]

            def epilogue(ps2, tglob, psplit=1):
                """LayerNorm of one [128, D] token tile living in 2 PSUM banks.

                psplit=2 runs two independent 64-partition chains (shorter
                serial latency; used for the very last tile)."""
                for ip in range(psplit):
                    pr = slice(ip * (128 // psplit), (ip + 1) * (128 // psplit))
                    _epilogue_part(ps2, tglob, pr, ip)

            def _epilogue_part(ps2, tglob, pr, ip):
                np_ = pr.stop - pr.start
                stats = st.tile([128, 2, 6], F32, tag="stats")
                mv = st.tile([128, 2], F32, tag="mv")
                std = st.tile([128, 1], F32, tag="std")
                rstd = st.tile([128, 1], F32, tag="rstd")
                shift = st.tile([128, 1], F32, tag="shift")

                for n in range(2):
                    nc.vector.bn_stats(stats[pr, n, :], ps2[n][pr, :])
                nc.vector.bn_aggr(mv[pr, :], stats[pr, :, :])
                # std = sqrt(var + eps); rstd = 1/std; shift = -mu * rstd
                nc.scalar.activation(std[pr, :], mv[pr, 1:2], AF.Sqrt, bias=eps_t[pr, :])
                nc.vector.reciprocal(rstd[pr, :], std[pr, :])
                nc.vector.scalar_tensor_tensor(
                    shift[pr, :], mv[pr, 0:1], -1.0, rstd[pr, :],
                    op0=OP.mult, op1=OP.mult,
                )
                yn = sp.tile([128, D], F32, tag="yn", name=f"yn_{ip}")
                rows = out_d[tglob * 128 : (tglob + 1) * 128, :]
                for n in range(2):
                    sl = slice(n * 512, (n + 1) * 512)
                    nc.scalar.activation(
                        yn[pr, sl],
                        ps2[n][pr, :],
                        AF.Identity,
                        bias=shift[pr, :],
                        scale=rstd[pr, :],
                    )
                    if with_affine:
                        og = sp.tile([128, 512], F32, tag="og", name=f"og_{ip}")
                        nc.vector.scalar_tensor_tensor(
                            og[pr, :], yn[pr, sl], 1.0, gb_sb[pr, 0:D][:, sl],
                            op0=OP.mult, op1=OP.mult,
                        )
                        nc.vector.tensor_add(
                            og[pr, :], og[pr, :], gb_sb[pr, D : 2 * D][:, sl]
                        )
                        nc.scalar.dma_start(rows[pr, sl], og[pr, :])
                    else:
                        nc.scalar.dma_start(rows[pr, sl], yn[pr, sl])

            def stage2_mouter(gt, cs, t0):
                """Y = G W2 for chunk 0: m(contraction)-outer over the first
                nt-1 token tiles so w2 can stream in; the last tile runs
                t-outer afterwards, releasing most PSUM banks early."""
                nt = cs // 128
                nm = nt - 1 if nt > 1 else nt
                ps2 = [
                    [ps.tile([128, 512], F32, tag="ps", name=f"ps2_{t}_{n}") for n in range(2)]
                    for t in range(nm)
                ]
                for m in range(MH):
                    for t in range(nm):
                        lhsT = gt[m][:, t * 128 : (t + 1) * 128]
                        for n in range(2):
                            nc.tensor.matmul(
                                ps2[t][n][:],
                                lhsT,
                                w2_slice(m, n),
                                start=(m == 0),
                                stop=(m == MH - 1),
                            )
                for t in range(nm):
                    epilogue(ps2[t], t0 + t)
                for t in range(nm, nt):
                    psl = [ps.tile([128, 512], F32, tag="ps", name=f"ps2l_{t}_{n}") for n in range(2)]
                    for m in range(MH):
                        lhsT = gt[m][:, t * 128 : (t + 1) * 128]
                        for n in range(2):
                            nc.tensor.matmul(
                                psl[n][:],
                                lhsT,
                                w2_slice(m, n),
                                start=(m == 0),
                                stop=(m == MH - 1),
                            )
                    epilogue(psl, t0 + t)

            def stage2_touter(gt, cs, t0, last=False):
                """Y = G W2, t-outer: low PSUM pressure, w2 already resident."""
                nt = cs // 128
                for t in range(nt):
                    ps2 = [ps.tile([128, 512], F32, tag="ps", name=f"ps2t_{t}_{n}") for n in range(2)]
                    for m in range(MH):
                        lhsT = gt[m][:, t * 128 : (t + 1) * 128]
                        for n in range(2):
                            nc.tensor.matmul(
                                ps2[n][:],
                                lhsT,
                                w2_slice(m, n),
                                start=(m == 0),
                                stop=(m == MH - 1),
                            )
                    epilogue(ps2, t0 + t)

            if len(chunks) > 1:
                xt_next = load_xt(1, offs[1], chunks[1])
            stage2_mouter(gt, chunks[0], 0)

            for ci in range(1, len(chunks)):
                cs = chunks[ci]
                gt = stage1(xt_next, cs)
                if ci + 1 < len(chunks):
                    xt_next = load_xt(ci + 1, offs[ci + 1], chunks[ci + 1])
                stage2_touter(
                    gt, cs, offs[ci] // 128, last=(ci == len(chunks) - 1)
                )

            rep_ctx.__exit__(None, None, None)

    nc.compile()
    return nc


_PROGRAM_CACHE = {}


def _get_program(C, with_affine):
    key = (C, with_affine)
    if key not in _PROGRAM_CACHE:
        _PROGRAM_CACHE[key] = build_program(C, with_affine)
    return _PROGRAM_CACHE[key]


def prepare(x, gate_w, expert_w1, expert_w2, ln_gamma, ln_beta):
    """Host-side routing + sharding. Returns (nc, in_maps, meta)."""
    x = np.asarray(x, dtype=np.float32)
    gate_w = np.asarray(gate_w, dtype=np.float32)
    expert_w1 = np.asarray(expert_w1, dtype=np.float32)
    expert_w2 = np.asarray(expert_w2, dtype=np.float32)
    ln_gamma = np.asarray(ln_gamma, dtype=np.float32)
    ln_beta = np.asarray(ln_beta, dtype=np.float32)

    B, T, _ = x.shape
    N = B * T
    xf = x.reshape(N, D)

    # Top-1 routing on host, in f64 (the f32 top-2 logit gap of this problem's
    # inputs is >8e-5, far above any rounding ambiguity).
    logits = xf.astype(np.float64) @ gate_w.astype(np.float64).T
    top = np.argmax(logits, axis=1)
    idx = [np.nonzero(top == e)[0] for e in range(E)]
    counts = [len(i) for i in idx]

    C = max(256, -(-max(counts) // 128) * 128)

    with_affine = not (np.all(ln_gamma == 1.0) and np.all(ln_beta == 0.0))
    nc = _get_program(C, with_affine)

    if with_affine:
        gb = np.empty((128, 2 * D), np.float32)
        gb[:, :D] = ln_gamma
        gb[:, D:] = ln_beta

    in_maps = []
    for e in range(E):
        xp = np.zeros((C, D), np.float32)
        xp[: counts[e]] = xf[idx[e]]
        xt = np.ascontiguousarray(
            xp.T.reshape(KD, 128, C).transpose(1, 0, 2)
        )  # [128, KD, C]; xt[p,k,t] = xp[t, k*128+p]
        w1 = np.ascontiguousarray(expert_w1[e].reshape(KD, 128, H))
        w2 = np.ascontiguousarray(
            expert_w2[e].reshape(KD, 2, 128, D).transpose(0, 2, 1, 3).reshape(KD, 128, H)
        )  # tile j holds m-tiles 2j | 2j+1 side by side
        m = {"xt": xt, "w1": w1, "w2": w2}
        if with_affine:
            m["gb"] = gb
        in_maps.append(m)

    return nc, in_maps, (idx, counts, B, T, N)


def assemble(results, meta):
    idx, counts, B, T, N = meta
    out = np.empty((N, D), np.float32)
    for e in range(E):
        out[idx[e]] = results[e]["out"][: counts[e]]
    return out.reshape(B, T, D)


def kernel(x, gate_w, expert_w1, expert_w2, ln_gamma, ln_beta):
    nc, in_maps, meta = prepare(x, gate_w, expert_w1, expert_w2, ln_gamma, ln_beta)
    res = run_bass_kernel_spmd(nc, in_maps, core_ids=list(range(E)))
    return assemble(res.results, meta)


# revision 19
# speedup vs baseline: 1.1504x; 1.1504x over previous
"""MoE top-1 routing + expert MLP + LayerNorm on 8 Trainium2 NeuronCores.

Expert-parallel: core e holds expert e's weights (E == n_cores == 8).
Host computes the (cheap) top-1 gate routing and does the token
dispatch/undispatch as part of sharding; each core runs its expert's MLP
(fp32r matmuls, exact gelu) and the final LayerNorm on its own tokens.

The top_val gate scaling of the reference is a mathematical no-op here:
LayerNorm is scale-invariant per token up to the eps placement
(top_val in [1-8e-8, 1)), so it is skipped (validated: rel err 6e-5).

Self-contained: hardcodes B=4, T=2048, D=1024, H=2048, E=8.
"""

import sys

sys.path.insert(0, "/opt/trn_rl_repo")

import numpy as np

import concourse.bass as bass
import concourse.bacc as bacc
import concourse.mybir as mybir
import concourse.tile as tile
from concourse.bass_utils import run_bass_kernel_spmd

D = 1024
H = 2048
E = 8
LN_EPS = 1e-5

F32 = mybir.dt.float32
F32R = mybir.dt.float32r
AF = mybir.ActivationFunctionType
OP = mybir.AluOpType

KD = D // 128   # 8  k-tiles of the d_model contraction
MH = H // 128   # 16 h-tiles of the hidden dim

# fp16 matmul operands halve the weight/activation DMA (the kernel's main
# stall source) at the same PE speed; PSUM/LayerNorm stay f32. Mixing 16/32
# bit matmul inputs is rejected by walrus, so it's all-or-nothing per
# matmul: False -> fp32r everywhere (rel err 2.1e-4), True -> ~5e-4.
DEFAULT_FP16 = True


def _chunk_sizes(C, cap):
    """Split C (multiple of 128, >=256) into chunks of 256..cap, mult of 128.

    cap=512 puts a big chunk first (more PE work to overlap the weight-DMA
    window at kernel start); cap=384 keeps pools small when SBUF is tight."""
    if cap >= 512:
        sizes = []
        rem = C
        while rem > 0:
            take = rem if rem <= 512 else (384 if rem == 640 else 512)
            sizes.append(take)
            rem -= take
    else:
        nch = -(-C // cap)
        base = (C // nch) // 128 * 128
        sizes = [base] * nch
        rem = (C - base * nch) // 128
        for i in range(rem):
            sizes[i] += 128
    assert sum(sizes) == C and all(
        c % 128 == 0 and 256 <= c <= 512 for c in sizes
    ), sizes
    return sizes


def build_program(C, with_affine, act=None, repeat=1, wdt=None):
    """Build the per-core Bass program: C tokens through one expert + LN."""
    if act is None:
        act = AF.Gelu
    if wdt is None:
        wdt = mybir.dt.float16 if DEFAULT_FP16 else F32R
    # the gb tile (8 KB/partition) pushes the 512-chunk layout over SBUF
    chunks = _chunk_sizes(C, cap=384 if with_affine else 512)
    nc = bacc.Bacc("TRN2", target_bir_lowering=False, debug=False, num_devices=E)

    xt_d = nc.dram_tensor("xt", [128, KD, C], wdt, kind="ExternalInput")
    w1_d = nc.dram_tensor("w1", [KD, 128, H], wdt, kind="ExternalInput")
    w2_d = nc.dram_tensor("w2", [KD, 128, H], wdt, kind="ExternalInput")
    if with_affine:
        gb_d = nc.dram_tensor("gb", [128, 2 * D], F32, kind="ExternalInput")
    out_d = nc.dram_tensor("out", [C, D], F32, kind="ExternalOutput")

    with tile.TileContext(nc) as tc:
        with (
            tc.tile_pool(name="wts", bufs=1) as wts,
            tc.tile_pool(name="xp", bufs=2) as xp,
            tc.tile_pool(name="gp", bufs=MH) as gp,
            tc.tile_pool(name="sp", bufs=2) as sp,
            tc.tile_pool(name="st", bufs=2) as st,
            tc.tile_pool(name="ps", bufs=8, space=bass.MemorySpace.PSUM) as ps,
        ):
            import contextlib

            rep_ctx = (
                tc.For_i(
                    0, repeat, 1, name="rep",
                    hint_engines=(mybir.EngineType.PE,),
                )
                if repeat > 1
                else contextlib.nullcontext()
            )

            # ---- chunk-0 tokens first so stage 1 can start ASAP
            def load_xt(ci, c0, cs):
                t = xp.tile([128, KD, cs], wdt, tag="xt", name=f"xt_{ci}")
                nc.sync.dma_start(t[:], xt_d[:, :, c0 : c0 + cs])
                return t

            offs = np.cumsum([0] + chunks).tolist()

            # constants + HAM-warmup garbage tiles live outside the bench loop
            eps_t = wts.tile([128, 1], F32, tag="eps")
            nc.vector.memset(eps_t[:], LN_EPS)
            warm_sb = wts.tile([128, 512], wdt, tag="warm")
            nc.vector.memset(warm_sb[:], 0.0)

            rep_ctx.__enter__()

            # dummy matmuls: keep the PE busy (and the HAM clock-gate warm)
            # while the first weight/activation DMAs land
            warm_ps = ps.tile([128, 384], F32, tag="ps", name="warm_ps")
            for i in range(16):
                nc.tensor.matmul(
                    warm_ps[:],
                    warm_sb[:, 0:128],
                    warm_sb[:, 128:512],
                    start=True,
                    stop=True,
                )

            # chunk-0 tokens + w1, interleaved per k-tile so stage 1 can
            # start after ~1 MB instead of ~10 MB
            cs0 = chunks[0]
            xt_sb = xp.tile([128, KD, cs0], wdt, tag="xt", name="xt_0")
            w1_sb = []
            for k in range(KD):
                nc.sync.dma_start(xt_sb[:, k, :], xt_d[:, k, 0:cs0])
                t = wts.tile([128, H], wdt, tag=f"w1_{k}", name=f"w1sb_{k}")
                nc.sync.dma_start(t[:], w1_d[k])
                w1_sb.append(t)

            def stage1(xt_t, cs):
                """G^T = gelu(W1^T X^T) for one token chunk. Returns 16 [128,cs] tiles."""
                gt = [None] * MH
                for half in range(2):
                    pst = [ps.tile([128, cs], F32, tag="ps", name=f"ps1_{half}_{i}") for i in range(8)]
                    for k in range(KD):
                        rhs = xt_t[:, k, :]
                        for h8 in range(8):
                            h = half * 8 + h8
                            nc.tensor.matmul(
                                pst[h8][:],
                                w1_sb[k][:, h * 128 : (h + 1) * 128],
                                rhs,
                                start=(k == 0),
                                stop=(k == KD - 1),
                            )
                    for h8 in range(8):
                        h = half * 8 + h8
                        g = gp.tile([128, cs], wdt, tag="gt", name=f"gt_{h}")
                        if act == "erf":
                            # exact gelu: 0.5 * x * (1 + erf(x/sqrt(2)))
                            e = sp.tile([128, cs], F32, tag="erf", name=f"erf_{h}", bufs=2)
                            nc.scalar.activation(
                                e[:], pst[h8][:], AF.Erf, scale=0.7071067811865476
                            )
                            u = sp.tile([128, cs], F32, tag="erf", name=f"erfu_{h}", bufs=2)
                            nc.vector.scalar_tensor_tensor(
                                u[:], e[:], 1.0, pst[h8][:],
                                op0=OP.add, op1=OP.mult,
                            )
                            nc.vector.tensor_scalar_mul(g[:], u[:], 0.5)
                        else:
                            nc.scalar.activation(g[:], pst[h8][:], act)
                        gt[h] = g
                return gt

            gt = stage1(xt_sb, cs0)

            # ---- second-phase weights (and LN affine), behind w1 in queue order
            w2_sb = []
            for j in range(KD):
                t = wts.tile([128, H], wdt, tag=f"w2_{j}", name=f"w2sb_{j}")
                nc.sync.dma_start(t[:], w2_d[j])
                w2_sb.append(t)
            if with_affine:
                gb_sb = wts.tile([128, 2 * D], F32, tag="gb")
                nc.sync.dma_start(gb_sb[:], gb_d[:])

            def w2_slice(m, n):
                # W2 m-tile [128, D] packed in pairs: tile j=m//2, cols (m%2)*D
                j, r = divmod(m, 2)
                return w2_sb[j][:, r * D + n * 512 : r * D + (n + 1) * 512]

            def epilogue(ps2, tglob, psplit=1):
                """LayerNorm of one [128, D] token tile living in 2 PSUM banks.

                psplit=2 runs two independent 64-partition chains (shorter
                serial latency; used for the very last tile)."""
                for ip in range(psplit):
                    pr = slice(ip * (128 // psplit), (ip + 1) * (128 // psplit))
                    _epilogue_part(ps2, tglob, pr, ip)

            def _epilogue_part(ps2, tglob, pr, ip):
                np_ = pr.stop - pr.start
                stats = st.tile([128, 2, 6], F32, tag="stats")
                mv = st.tile([128, 2], F32, tag="mv")
                std = st.tile([128, 1], F32, tag="std")
                rstd = st.tile([128, 1], F32, tag="rstd")
                shift = st.tile([128, 1], F32, tag="shift")

                for n in range(2):
                    nc.vector.bn_stats(stats[pr, n, :], ps2[n][pr, :])
                nc.vector.bn_aggr(mv[pr, :], stats[pr, :, :])
                # std = sqrt(var + eps); rstd = 1/std; shift = -mu * rstd
                nc.scalar.activation(std[pr, :], mv[pr, 1:2], AF.Sqrt, bias=eps_t[pr, :])
                nc.vector.reciprocal(rstd[pr, :], std[pr, :])
                nc.vector.scalar_tensor_tensor(
                    shift[pr, :], mv[pr, 0:1], -1.0, rstd[pr, :],
                    op0=OP.mult, op1=OP.mult,
                )
                yn = sp.tile([128, D], F32, tag="yn", name=f"yn_{ip}")
                rows = out_d[tglob * 128 : (tglob + 1) * 128, :]
                for n in range(2):
                    sl = slice(n * 512, (n + 1) * 512)
                    nc.scalar.activation(
                        yn[pr, sl],
                        ps2[n][pr, :],
                        AF.Identity,
                        bias=shift[pr, :],
                        scale=rstd[pr, :],
                    )
                    if with_affine:
                        og = sp.tile([128, 512], F32, tag="og", name=f"og_{ip}")
                        nc.vector.scalar_tensor_tensor(
                            og[pr, :], yn[pr, sl], 1.0, gb_sb[pr, 0:D][:, sl],
                            op0=OP.mult, op1=OP.mult,
                        )
                        nc.vector.tensor_add(
                            og[pr, :], og[pr, :], gb_sb[pr, D : 2 * D][:, sl]
                        )
                        nc.scalar.dma_start(rows[pr, sl], og[pr, :])
                    else:
                        nc.scalar.dma_start(rows[pr, sl], yn[pr, sl])

            def stage2_mouter(gt, cs, t0):
                """Y = G W2 for chunk 0: m(contraction)-outer over the first
                nt-1 token tiles so w2 can stream in; the last tile runs
                t-outer afterwards, releasing most PSUM banks early."""
                nt = cs // 128
                nm = nt - 1 if nt > 1 else nt
                ps2 = [
                    [ps.tile([128, 512], F32, tag="ps", name=f"ps2_{t}_{n}") for n in range(2)]
                    for t in range(nm)
                ]
                for m in range(MH):
                    for t in range(nm):
                        lhsT = gt[m][:, t * 128 : (t + 1) * 128]
                        for n in range(2):
                            nc.tensor.matmul(
                                ps2[t][n][:],
                                lhsT,
                                w2_slice(m, n),
                                start=(m == 0),
                                stop=(m == MH - 1),
                            )
                for t in range(nm):
                    epilogue(ps2[t], t0 + t)
                for t in range(nm, nt):
                    psl = [ps.tile([128, 512], F32, tag="ps", name=f"ps2l_{t}_{n}") for n in range(2)]
                    for m in range(MH):
                        lhsT = gt[m][:, t * 128 : (t + 1) * 128]
                        for n in range(2):
                            nc.tensor.matmul(
                                psl[n][:],
                                lhsT,
                                w2_slice(m, n),
                                start=(m == 0),
                                stop=(m == MH - 1),
                            )
                    epilogue(psl, t0 + t)

            def stage2_touter(gt, cs, t0, last=False):
                """Y = G W2, t-outer: low PSUM pressure, w2 already resident."""
                nt = cs // 128
                for t in range(nt):
                    ps2 = [ps.tile([128, 512], F32, tag="ps", name=f"ps2t_{t}_{n}") for n in range(2)]
                    for m in range(MH):
                        lhsT = gt[m][:, t * 128 : (t + 1) * 128]
                        for n in range(2):
                            nc.tensor.matmul(
                                ps2[n][:],
                                lhsT,
                                w2_slice(m, n),
                                start=(m == 0),
                                stop=(m == MH - 1),
                            )
                    epilogue(ps2, t0 + t)

            if len(chunks) > 1:
                xt_next = load_xt(1, offs[1], chunks[1])
            stage2_mouter(gt, chunks[0], 0)

            for ci in range(1, len(chunks)):
                cs = chunks[ci]
                gt = stage1(xt_next, cs)
                if ci + 1 < len(chunks):
                    xt_next = load_xt(ci + 1, offs[ci + 1], chunks[ci + 1])
                stage2_touter(
                    gt, cs, offs[ci] // 128, last=(ci == len(chunks) - 1)
                )

            rep_ctx.__exit__(None, None, None)

    nc.compile()
    return nc


_PROGRAM_CACHE = {}


def _get_program(C, with_affine):
    key = (C, with_affine)
    if key not in _PROGRAM_CACHE:
        _PROGRAM_CACHE[key] = build_program(C, with_affine)
    return _PROGRAM_CACHE[key]


def prepare(x, gate_w, expert_w1, expert_w2, ln_gamma, ln_beta):
    """Host-side routing + sharding. Returns (nc, in_maps, meta)."""
    x = np.asarray(x, dtype=np.float32)
    gate_w = np.asarray(gate_w, dtype=np.float32)
    expert_w1 = np.asarray(expert_w1, dtype=np.float32)
    expert_w2 = np.asarray(expert_w2, dtype=np.float32)
    ln_gamma = np.asarray(ln_gamma, dtype=np.float32)
    ln_beta = np.asarray(ln_beta, dtype=np.float32)

    B, T, _ = x.shape
    N = B * T
    xf = x.reshape(N, D)

    # Top-1 routing on host, in f64 (the f32 top-2 logit gap of this problem's
    # inputs is >8e-5, far above any rounding ambiguity).
    logits = xf.astype(np.float64) @ gate_w.astype(np.float64).T
    top = np.argmax(logits, axis=1)
    idx = [np.nonzero(top == e)[0] for e in range(E)]
    counts = [len(i) for i in idx]

    C = max(256, -(-max(counts) // 128) * 128)

    with_affine = not (np.all(ln_gamma == 1.0) and np.all(ln_beta == 0.0))
    nc = _get_program(C, with_affine)

    if with_affine:
        gb = np.empty((128, 2 * D), np.float32)
        gb[:, :D] = ln_gamma
        gb[:, D:] = ln_beta

    in_maps = []
    for e in range(E):
        npdt = np.float16 if DEFAULT_FP16 else np.float32
        xp = np.zeros((C, D), npdt)
        xp[: counts[e]] = xf[idx[e]]
        xt = np.ascontiguousarray(
            xp.T.reshape(KD, 128, C).transpose(1, 0, 2)
        )  # [128, KD, C]; xt[p,k,t] = xp[t, k*128+p]
        w1 = np.ascontiguousarray(expert_w1[e].reshape(KD, 128, H), dtype=npdt)
        w2 = np.ascontiguousarray(
            expert_w2[e]
            .reshape(KD, 2, 128, D)
            .transpose(0, 2, 1, 3)
            .reshape(KD, 128, H),
            dtype=npdt,
        )  # tile j holds m-tiles 2j | 2j+1 side by side
        m = {"xt": xt, "w1": w1, "w2": w2}
        if with_affine:
            m["gb"] = gb
        in_maps.append(m)

    return nc, in_maps, (idx, counts, B, T, N)


def assemble(results, meta):
    idx, counts, B, T, N = meta
    out = np.empty((N, D), np.float32)
    for e in range(E):
        out[idx[e]] = results[e]["out"][: counts[e]]
    return out.reshape(B, T, D)


def kernel(x, gate_w, expert_w1, expert_w2, ln_gamma, ln_beta):
    nc, in_maps, meta = prepare(x, gate_w, expert_w1, expert_w2, ln_gamma, ln_beta)
    res = run_bass_kernel_spmd(nc, in_maps, core_ids=list(range(E)))
    return assemble(res.results, meta)


# revision 20
# speedup vs baseline: 1.1689x; 1.0160x over previous
"""MoE top-1 routing + expert MLP + LayerNorm on 8 Trainium2 NeuronCores.

Expert-parallel: core e holds expert e's weights (E == n_cores == 8).
Host computes the (cheap) top-1 gate routing and does the token
dispatch/undispatch as part of sharding; each core runs its expert's MLP
(fp32r matmuls, exact gelu) and the final LayerNorm on its own tokens.

The top_val gate scaling of the reference is a mathematical no-op here:
LayerNorm is scale-invariant per token up to the eps placement
(top_val in [1-8e-8, 1)), so it is skipped (validated: rel err 6e-5).

Self-contained: hardcodes B=4, T=2048, D=1024, H=2048, E=8.
"""

import sys

sys.path.insert(0, "/opt/trn_rl_repo")

import numpy as np

import concourse.bass as bass
import concourse.bacc as bacc
import concourse.mybir as mybir
import concourse.tile as tile
from concourse.bass_utils import run_bass_kernel_spmd

D = 1024
H = 2048
E = 8
LN_EPS = 1e-5

F32 = mybir.dt.float32
F32R = mybir.dt.float32r
AF = mybir.ActivationFunctionType
OP = mybir.AluOpType

KD = D // 128   # 8  k-tiles of the d_model contraction
MH = H // 128   # 16 h-tiles of the hidden dim

# fp16 matmul operands halve the weight/activation DMA (the kernel's main
# stall source) at the same PE speed; PSUM/LayerNorm stay f32. Mixing 16/32
# bit matmul inputs is rejected by walrus, so it's all-or-nothing per
# matmul: False -> fp32r everywhere (rel err 2.1e-4), True -> ~5e-4.
DEFAULT_FP16 = True


def _chunk_sizes(C, cap):
    """Split C (multiple of 128, >=256) into chunks of 256..cap, mult of 128.

    cap=512 puts a big chunk first (more PE work to overlap the weight-DMA
    window at kernel start); cap=384 keeps pools small when SBUF is tight."""
    if cap >= 512:
        sizes = []
        rem = C
        while rem > 0:
            take = rem if rem <= 512 else (384 if rem == 640 else 512)
            sizes.append(take)
            rem -= take
    else:
        nch = -(-C // cap)
        base = (C // nch) // 128 * 128
        sizes = [base] * nch
        rem = (C - base * nch) // 128
        for i in range(rem):
            sizes[i] += 128
    assert sum(sizes) == C and all(
        c % 128 == 0 and 256 <= c <= 512 for c in sizes
    ), sizes
    return sizes


def build_program(C, with_affine, act=None, repeat=1, wdt=None):
    """Build the per-core Bass program: C tokens through one expert + LN."""
    if act is None:
        act = AF.Gelu
    if wdt is None:
        wdt = mybir.dt.float16 if DEFAULT_FP16 else F32R
    # the gb tile (8 KB/partition) pushes the 512-chunk layout over SBUF
    chunks = _chunk_sizes(C, cap=384 if with_affine else 512)
    nc = bacc.Bacc("TRN2", target_bir_lowering=False, debug=False, num_devices=E)

    xt_d = nc.dram_tensor("xt", [128, KD, C], wdt, kind="ExternalInput")
    w1_d = nc.dram_tensor("w1", [KD, 128, H], wdt, kind="ExternalInput")
    w2_d = nc.dram_tensor("w2", [KD, 128, H], wdt, kind="ExternalInput")
    if with_affine:
        gb_d = nc.dram_tensor("gb", [128, 2 * D], F32, kind="ExternalInput")
    out_d = nc.dram_tensor("out", [C, D], F32, kind="ExternalOutput")

    with tile.TileContext(nc) as tc:
        with (
            tc.tile_pool(name="wts", bufs=1) as wts,
            tc.tile_pool(name="xp", bufs=2) as xp,
            tc.tile_pool(name="gp", bufs=MH) as gp,
            tc.tile_pool(name="sp", bufs=2) as sp,
            tc.tile_pool(name="st", bufs=2) as st,
            tc.tile_pool(name="ps", bufs=8, space=bass.MemorySpace.PSUM) as ps,
        ):
            import contextlib

            rep_ctx = (
                tc.For_i(
                    0, repeat, 1, name="rep",
                    hint_engines=(mybir.EngineType.PE,),
                )
                if repeat > 1
                else contextlib.nullcontext()
            )

            # ---- chunk-0 tokens first so stage 1 can start ASAP
            def load_xt(ci, c0, cs):
                t = xp.tile([128, KD, cs], wdt, tag="xt", name=f"xt_{ci}")
                nc.sync.dma_start(t[:], xt_d[:, :, c0 : c0 + cs])
                return t

            offs = np.cumsum([0] + chunks).tolist()

            # constants + HAM-warmup garbage tiles live outside the bench loop
            eps_t = wts.tile([128, 1], F32, tag="eps")
            nc.vector.memset(eps_t[:], LN_EPS)
            warm_sb = wts.tile([128, 512], wdt, tag="warm")
            nc.vector.memset(warm_sb[:].bitcast(mybir.dt.uint32), 0)

            rep_ctx.__enter__()

            # dummy matmuls: keep the PE busy (and the HAM clock-gate warm)
            # while the first weight/activation DMAs land
            warm_ps = ps.tile([128, 384], F32, tag="ps", name="warm_ps")
            for i in range(16):
                nc.tensor.matmul(
                    warm_ps[:],
                    warm_sb[:, 0:128],
                    warm_sb[:, 128:512],
                    start=True,
                    stop=True,
                )

            # chunk-0 tokens + w1, interleaved per k-tile so stage 1 can
            # start after ~1 MB instead of ~10 MB
            cs0 = chunks[0]
            xt_sb = xp.tile([128, KD, cs0], wdt, tag="xt", name="xt_0")
            w1_sb = []
            for k in range(KD):
                nc.sync.dma_start(xt_sb[:, k, :], xt_d[:, k, 0:cs0])
                t = wts.tile([128, H], wdt, tag=f"w1_{k}", name=f"w1sb_{k}")
                nc.sync.dma_start(t[:], w1_d[k])
                w1_sb.append(t)

            def stage1(xt_t, cs):
                """G^T = gelu(W1^T X^T) for one token chunk. Returns 16 [128,cs] tiles."""
                gt = [None] * MH
                for half in range(2):
                    pst = [ps.tile([128, cs], F32, tag="ps", name=f"ps1_{half}_{i}") for i in range(8)]
                    for k in range(KD):
                        rhs = xt_t[:, k, :]
                        for h8 in range(8):
                            h = half * 8 + h8
                            nc.tensor.matmul(
                                pst[h8][:],
                                w1_sb[k][:, h * 128 : (h + 1) * 128],
                                rhs,
                                start=(k == 0),
                                stop=(k == KD - 1),
                            )
                    for h8 in range(8):
                        h = half * 8 + h8
                        g = gp.tile([128, cs], wdt, tag="gt", name=f"gt_{h}")
                        if act == "erf":
                            # exact gelu: 0.5 * x * (1 + erf(x/sqrt(2)))
                            e = sp.tile([128, cs], F32, tag="erf", name=f"erf_{h}", bufs=2)
                            nc.scalar.activation(
                                e[:], pst[h8][:], AF.Erf, scale=0.7071067811865476
                            )
                            u = sp.tile([128, cs], F32, tag="erf", name=f"erfu_{h}", bufs=2)
                            nc.vector.scalar_tensor_tensor(
                                u[:], e[:], 1.0, pst[h8][:],
                                op0=OP.add, op1=OP.mult,
                            )
                            nc.vector.tensor_scalar_mul(g[:], u[:], 0.5)
                        else:
                            nc.scalar.activation(g[:], pst[h8][:], act)
                        gt[h] = g
                return gt

            gt = stage1(xt_sb, cs0)

            # ---- second-phase weights (and LN affine), behind w1 in queue order
            w2_sb = []
            for j in range(KD):
                t = wts.tile([128, H], wdt, tag=f"w2_{j}", name=f"w2sb_{j}")
                nc.sync.dma_start(t[:], w2_d[j])
                w2_sb.append(t)
            if with_affine:
                gb_sb = wts.tile([128, 2 * D], F32, tag="gb")
                nc.sync.dma_start(gb_sb[:], gb_d[:])

            def w2_slice(m, n):
                # W2 m-tile [128, D] packed in pairs: tile j=m//2, cols (m%2)*D
                j, r = divmod(m, 2)
                return w2_sb[j][:, r * D + n * 512 : r * D + (n + 1) * 512]

            def epilogue(ps2, tglob, psplit=1):
                """LayerNorm of one [128, D] token tile living in 2 PSUM banks.

                psplit=2 runs two independent 64-partition chains (shorter
                serial latency; used for the very last tile)."""
                for ip in range(psplit):
                    pr = slice(ip * (128 // psplit), (ip + 1) * (128 // psplit))
                    _epilogue_part(ps2, tglob, pr, ip)

            def _epilogue_part(ps2, tglob, pr, ip):
                np_ = pr.stop - pr.start
                stats = st.tile([128, 2, 6], F32, tag="stats")
                mv = st.tile([128, 2], F32, tag="mv")
                std = st.tile([128, 1], F32, tag="std")
                rstd = st.tile([128, 1], F32, tag="rstd")
                shift = st.tile([128, 1], F32, tag="shift")

                for n in range(2):
                    nc.vector.bn_stats(stats[pr, n, :], ps2[n][pr, :])
                nc.vector.bn_aggr(mv[pr, :], stats[pr, :, :])
                # std = sqrt(var + eps); rstd = 1/std; shift = -mu * rstd
                nc.scalar.activation(std[pr, :], mv[pr, 1:2], AF.Sqrt, bias=eps_t[pr, :])
                nc.vector.reciprocal(rstd[pr, :], std[pr, :])
                nc.vector.scalar_tensor_tensor(
                    shift[pr, :], mv[pr, 0:1], -1.0, rstd[pr, :],
                    op0=OP.mult, op1=OP.mult,
                )
                yn = sp.tile([128, D], F32, tag="yn", name=f"yn_{ip}")
                rows = out_d[tglob * 128 : (tglob + 1) * 128, :]
                for n in range(2):
                    sl = slice(n * 512, (n + 1) * 512)
                    nc.scalar.activation(
                        yn[pr, sl],
                        ps2[n][pr, :],
                        AF.Identity,
                        bias=shift[pr, :],
                        scale=rstd[pr, :],
                    )
                    if with_affine:
                        og = sp.tile([128, 512], F32, tag="og", name=f"og_{ip}")
                        nc.vector.scalar_tensor_tensor(
                            og[pr, :], yn[pr, sl], 1.0, gb_sb[pr, 0:D][:, sl],
                            op0=OP.mult, op1=OP.mult,
                        )
                        nc.vector.tensor_add(
                            og[pr, :], og[pr, :], gb_sb[pr, D : 2 * D][:, sl]
                        )
                        nc.scalar.dma_start(rows[pr, sl], og[pr, :])
                    else:
                        nc.scalar.dma_start(rows[pr, sl], yn[pr, sl])

            def stage2_mouter(gt, cs, t0):
                """Y = G W2 for chunk 0: m(contraction)-outer over the first
                nt-1 token tiles so w2 can stream in; the last tile runs
                t-outer afterwards, releasing most PSUM banks early."""
                nt = cs // 128
                nm = nt - 1 if nt > 1 else nt
                ps2 = [
                    [ps.tile([128, 512], F32, tag="ps", name=f"ps2_{t}_{n}") for n in range(2)]
                    for t in range(nm)
                ]
                for m in range(MH):
                    for t in range(nm):
                        lhsT = gt[m][:, t * 128 : (t + 1) * 128]
                        for n in range(2):
                            nc.tensor.matmul(
                                ps2[t][n][:],
                                lhsT,
                                w2_slice(m, n),
                                start=(m == 0),
                                stop=(m == MH - 1),
                            )
                for t in range(nm):
                    epilogue(ps2[t], t0 + t)
                for t in range(nm, nt):
                    psl = [ps.tile([128, 512], F32, tag="ps", name=f"ps2l_{t}_{n}") for n in range(2)]
                    for m in range(MH):
                        lhsT = gt[m][:, t * 128 : (t + 1) * 128]
                        for n in range(2):
                            nc.tensor.matmul(
                                psl[n][:],
                                lhsT,
                                w2_slice(m, n),
                                start=(m == 0),
                                stop=(m == MH - 1),
                            )
                    epilogue(psl, t0 + t)

            def stage2_touter(gt, cs, t0, last=False):
                """Y = G W2, t-outer: low PSUM pressure, w2 already resident."""
                nt = cs // 128
                for t in range(nt):
                    ps2 = [ps.tile([128, 512], F32, tag="ps", name=f"ps2t_{t}_{n}") for n in range(2)]
                    for m in range(MH):
                        lhsT = gt[m][:, t * 128 : (t + 1) * 128]
                        for n in range(2):
                            nc.tensor.matmul(
                                ps2[n][:],
                                lhsT,
                                w2_slice(m, n),
                                start=(m == 0),
                                stop=(m == MH - 1),
                            )
                    epilogue(ps2, t0 + t)

            if len(chunks) > 1:
                xt_next = load_xt(1, offs[1], chunks[1])
            stage2_mouter(gt, chunks[0], 0)

            for ci in range(1, len(chunks)):
                cs = chunks[ci]
                gt = stage1(xt_next, cs)
                if ci + 1 < len(chunks):
                    xt_next = load_xt(ci + 1, offs[ci + 1], chunks[ci + 1])
                stage2_touter(
                    gt, cs, offs[ci] // 128, last=(ci == len(chunks) - 1)
                )

            rep_ctx.__exit__(None, None, None)

    nc.compile()
    return nc


_PROGRAM_CACHE = {}


def _get_program(C, with_affine):
    key = (C, with_affine)
    if key not in _PROGRAM_CACHE:
        _PROGRAM_CACHE[key] = build_program(C, with_affine)
    return _PROGRAM_CACHE[key]


def prepare(x, gate_w, expert_w1, expert_w2, ln_gamma, ln_beta):
    """Host-side routing + sharding. Returns (nc, in_maps, meta)."""
    x = np.asarray(x, dtype=np.float32)
    gate_w = np.asarray(gate_w, dtype=np.float32)
    expert_w1 = np.asarray(expert_w1, dtype=np.float32)
    expert_w2 = np.asarray(expert_w2, dtype=np.float32)
    ln_gamma = np.asarray(ln_gamma, dtype=np.float32)
    ln_beta = np.asarray(ln_beta, dtype=np.float32)

    B, T, _ = x.shape
    N = B * T
    xf = x.reshape(N, D)

    # Top-1 routing on host, in f64 (the f32 top-2 logit gap of this problem's
    # inputs is >8e-5, far above any rounding ambiguity).
    logits = xf.astype(np.float64) @ gate_w.astype(np.float64).T
    top = np.argmax(logits, axis=1)
    idx = [np.nonzero(top == e)[0] for e in range(E)]
    counts = [len(i) for i in idx]

    C = max(256, -(-max(counts) // 128) * 128)

    with_affine = not (np.all(ln_gamma == 1.0) and np.all(ln_beta == 0.0))
    nc = _get_program(C, with_affine)

    if with_affine:
        gb = np.empty((128, 2 * D), np.float32)
        gb[:, :D] = ln_gamma
        gb[:, D:] = ln_beta

    in_maps = []
    for e in range(E):
        npdt = np.float16 if DEFAULT_FP16 else np.float32
        xp = np.zeros((C, D), npdt)
        xp[: counts[e]] = xf[idx[e]]
        xt = np.ascontiguousarray(
            xp.T.reshape(KD, 128, C).transpose(1, 0, 2)
        )  # [128, KD, C]; xt[p,k,t] = xp[t, k*128+p]
        w1 = np.ascontiguousarray(expert_w1[e].reshape(KD, 128, H), dtype=npdt)
        w2 = np.ascontiguousarray(
            expert_w2[e]
            .reshape(KD, 2, 128, D)
            .transpose(0, 2, 1, 3)
            .reshape(KD, 128, H),
            dtype=npdt,
        )  # tile j holds m-tiles 2j | 2j+1 side by side
        m = {"xt": xt, "w1": w1, "w2": w2}
        if with_affine:
            m["gb"] = gb
        in_maps.append(m)

    return nc, in_maps, (idx, counts, B, T, N)


def assemble(results, meta):
    idx, counts, B, T, N = meta
    out = np.empty((N, D), np.float32)
    for e in range(E):
        out[idx[e]] = results[e]["out"][: counts[e]]
    return out.reshape(B, T, D)


def kernel(x, gate_w, expert_w1, expert_w2, ln_gamma, ln_beta):
    nc, in_maps, meta = prepare(x, gate_w, expert_w1, expert_w2, ln_gamma, ln_beta)
    res = run_bass_kernel_spmd(nc, in_maps, core_ids=list(range(E)))
    return assemble(res.results, meta)


# revision 23
# speedup vs baseline: 1.2422x; 1.0627x over previous
"""MoE top-1 routing + expert MLP + LayerNorm on 8 Trainium2 NeuronCores.

Expert-parallel (E == n_cores == 8): core e holds expert e's weights. The
host computes the (negligible, 67 MFLOP) top-1 gate routing in f64 — the
top-2 logit gap of these inputs is >8e-5, far above f32 rounding, so the
routing matches the reference bit-for-bit — and performs the token
dispatch/undispatch as part of the input sharding / output gather.

Per core (C = max expert load padded to 128, here 1152 tokens):
  stage 1  A^T = W1^T X^T   PE, fp16 operands, f32 PSUM accumulate
           G^T = gelu(A^T)  ScalarE LUT
  stage 2  Y   = G W2       PE (G^T token-slices are the stationary operand,
                            so Y lands token-major for the LayerNorm)
  epilogue LayerNorm        bn_stats/bn_aggr on VectorE + per-partition
                            scale/bias activation on ScalarE, all f32
Token chunks are pipelined so the w1 stream overlaps stage 1 of chunk 0
(k-outer contraction), w2 streams during stage-2-of-chunk-0 (m-outer), and
16 warmup matmuls keep the PE HAM clock-gate warm through the initial DMA.

The top_val gate scale of the reference is a mathematical no-op: LayerNorm
is scale-invariant per token up to eps placement (top_val in [1-8e-8, 1)),
so it is skipped. fp16 operands (f32 accumulate) keep rel err at 4.2e-4
while halving the weight-DMA, which bounds the kernel's start-up phase;
the PE multiplies bf16/fp16/fp32r all at 1 col/cycle, and the fp32r path
(DEFAULT_FP16=False) measures ~10% slower at 2.1e-4 rel err.

Measured: ~155-165 us/core steady state on HW (cost model: 137 us;
PE-streaming floor for this shape is ~124 us).

Self-contained: hardcodes D=1024, H=2048, E=8; C adapts to routing counts.
"""

import sys

sys.path.insert(0, "/opt/trn_rl_repo")

import numpy as np

import concourse.bass as bass
import concourse.bacc as bacc
import concourse.mybir as mybir
import concourse.tile as tile
from concourse.bass_utils import run_bass_kernel_spmd

D = 1024
H = 2048
E = 8
LN_EPS = 1e-5

F32 = mybir.dt.float32
F32R = mybir.dt.float32r
AF = mybir.ActivationFunctionType
OP = mybir.AluOpType

KD = D // 128   # 8  k-tiles of the d_model contraction
MH = H // 128   # 16 h-tiles of the hidden dim

# fp16 matmul operands halve the weight/activation DMA (the kernel's main
# stall source) at the same PE speed; PSUM/LayerNorm stay f32. Mixing 16/32
# bit matmul inputs is rejected by walrus, so it's all-or-nothing per
# matmul: False -> fp32r everywhere (rel err 2.1e-4), True -> ~5e-4.
DEFAULT_FP16 = True


def _chunk_tiles(C, cap):
    """Split C into chunks of token tiles: full 128-token tiles plus one
    ragged tail tile (C % 128) so no capacity padding is computed. Each
    chunk's column count stays <= cap (the PSUM bank / moving-dim limit).

    cap=512 puts a big chunk first (more PE work to overlap the weight-DMA
    window at kernel start); cap=384 keeps pools small when SBUF is tight."""
    tiles = [128] * (C // 128)
    if C % 128:
        tiles.append(C % 128)
    # distribute tiles over the minimum number of chunks, front-loaded so
    # chunk 0 is big (overlaps the weight-DMA window) and no chunk is tiny
    ntpc = cap // 128
    nch = -(-len(tiles) // ntpc)
    per = [len(tiles) // nch] * nch
    for i in range(len(tiles) - sum(per)):
        per[i] += 1
    chunks = []
    it = iter(tiles)
    for n in per:
        chunks.append([next(it) for _ in range(n)])
    assert sum(sum(c) for c in chunks) == C
    return chunks


def build_program(C, with_affine, act=None, repeat=1, wdt=None):
    """Build the per-core Bass program: C tokens through one expert + LN."""
    if act is None:
        act = AF.Gelu
    if wdt is None:
        wdt = mybir.dt.float16 if DEFAULT_FP16 else F32R
    # the gb tile (8 KB/partition) pushes the 512-chunk layout over SBUF
    tchunks = _chunk_tiles(C, cap=384 if with_affine else 512)
    chunks = [sum(c) for c in tchunks]
    nc = bacc.Bacc("TRN2", target_bir_lowering=False, debug=False, num_devices=E)

    xt_d = nc.dram_tensor("xt", [128, KD, C], wdt, kind="ExternalInput")
    w1_d = nc.dram_tensor("w1", [KD, 128, H], wdt, kind="ExternalInput")
    w2_d = nc.dram_tensor("w2", [KD, 128, H], wdt, kind="ExternalInput")
    if with_affine:
        gb_d = nc.dram_tensor("gb", [128, 2 * D], F32, kind="ExternalInput")
    out_d = nc.dram_tensor("out", [C, D], F32, kind="ExternalOutput")

    with tile.TileContext(nc) as tc:
        with (
            tc.tile_pool(name="wts", bufs=1) as wts,
            tc.tile_pool(name="xp", bufs=2) as xp,
            tc.tile_pool(name="gp", bufs=MH) as gp,
            tc.tile_pool(name="sp", bufs=2) as sp,
            tc.tile_pool(name="st", bufs=2) as st,
            tc.tile_pool(name="ps", bufs=8, space=bass.MemorySpace.PSUM) as ps,
        ):
            import contextlib

            rep_ctx = (
                tc.For_i(
                    0, repeat, 1, name="rep",
                    hint_engines=(mybir.EngineType.PE,),
                )
                if repeat > 1
                else contextlib.nullcontext()
            )

            # ---- chunk-0 tokens first so stage 1 can start ASAP
            def load_xt(ci, c0, cs):
                t = xp.tile([128, KD, cs], wdt, tag="xt", name=f"xt_{ci}")
                nc.sync.dma_start(t[:], xt_d[:, :, c0 : c0 + cs])
                return t

            offs = np.cumsum([0] + chunks).tolist()

            # constants + HAM-warmup garbage tiles live outside the bench loop
            eps_t = wts.tile([128, 1], F32, tag="eps")
            nc.vector.memset(eps_t[:], LN_EPS)
            warm_sb = wts.tile([128, 512], wdt, tag="warm")
            nc.vector.memset(warm_sb[:].bitcast(mybir.dt.uint32), 0)

            rep_ctx.__enter__()

            # dummy matmuls: keep the PE busy (and the HAM clock-gate warm)
            # while the first weight/activation DMAs land
            warm_ps = ps.tile([128, 384], F32, tag="ps", name="warm_ps")
            for i in range(16):
                nc.tensor.matmul(
                    warm_ps[:],
                    warm_sb[:, 0:128],
                    warm_sb[:, 128:512],
                    start=True,
                    stop=True,
                )

            # chunk-0 tokens + w1, interleaved per k-tile so stage 1 can
            # start after ~1 MB instead of ~10 MB
            cs0 = chunks[0]
            xt_sb = xp.tile([128, KD, cs0], wdt, tag="xt", name="xt_0")
            w1_sb = []
            for k in range(KD):
                nc.sync.dma_start(xt_sb[:, k, :], xt_d[:, k, 0:cs0])
                t = wts.tile([128, H], wdt, tag=f"w1_{k}", name=f"w1sb_{k}")
                nc.sync.dma_start(t[:], w1_d[k])
                w1_sb.append(t)

            def stage1(xt_t, cs):
                """G^T = gelu(W1^T X^T) for one token chunk. Returns 16 [128,cs] tiles."""
                gt = [None] * MH
                for half in range(2):
                    pst = [ps.tile([128, cs], F32, tag="ps", name=f"ps1_{half}_{i}") for i in range(8)]
                    for k in range(KD):
                        rhs = xt_t[:, k, :]
                        for h8 in range(8):
                            h = half * 8 + h8
                            nc.tensor.matmul(
                                pst[h8][:],
                                w1_sb[k][:, h * 128 : (h + 1) * 128],
                                rhs,
                                start=(k == 0),
                                stop=(k == KD - 1),
                            )
                    for h8 in range(8):
                        h = half * 8 + h8
                        g = gp.tile([128, cs], wdt, tag="gt", name=f"gt_{h}")
                        if act == "erf":
                            # exact gelu: 0.5 * x * (1 + erf(x/sqrt(2)))
                            e = sp.tile([128, cs], F32, tag="erf", name=f"erf_{h}", bufs=2)
                            nc.scalar.activation(
                                e[:], pst[h8][:], AF.Erf, scale=0.7071067811865476
                            )
                            u = sp.tile([128, cs], F32, tag="erf", name=f"erfu_{h}", bufs=2)
                            nc.vector.scalar_tensor_tensor(
                                u[:], e[:], 1.0, pst[h8][:],
                                op0=OP.add, op1=OP.mult,
                            )
                            nc.vector.tensor_scalar_mul(g[:], u[:], 0.5)
                        else:
                            nc.scalar.activation(g[:], pst[h8][:], act)
                        gt[h] = g
                return gt

            gt = stage1(xt_sb, cs0)

            # ---- second-phase weights (and LN affine), behind w1 in queue order
            w2_sb = []
            for j in range(KD):
                t = wts.tile([128, H], wdt, tag=f"w2_{j}", name=f"w2sb_{j}")
                nc.sync.dma_start(t[:], w2_d[j])
                w2_sb.append(t)
            if with_affine:
                gb_sb = wts.tile([128, 2 * D], F32, tag="gb")
                nc.sync.dma_start(gb_sb[:], gb_d[:])

            def w2_slice(m, n):
                # W2 m-tile [128, D] packed in pairs: tile j=m//2, cols (m%2)*D
                j, r = divmod(m, 2)
                return w2_sb[j][:, r * D + n * 512 : r * D + (n + 1) * 512]

            def epilogue(ps2, tok0, tsz):
                """LayerNorm of one [tsz<=128, D] token tile in 2 PSUM banks."""
                pr = slice(0, tsz)
                ip = 0
                stats = st.tile([128, 2, 6], F32, tag="stats")
                mv = st.tile([128, 2], F32, tag="mv")
                std = st.tile([128, 1], F32, tag="std")
                rstd = st.tile([128, 1], F32, tag="rstd")
                shift = st.tile([128, 1], F32, tag="shift")

                for n in range(2):
                    nc.vector.bn_stats(stats[pr, n, :], ps2[n][pr, :])
                nc.vector.bn_aggr(mv[pr, :], stats[pr, :, :])
                # std = sqrt(var + eps); rstd = 1/std; shift = -mu * rstd
                nc.scalar.activation(std[pr, :], mv[pr, 1:2], AF.Sqrt, bias=eps_t[pr, :])
                nc.vector.reciprocal(rstd[pr, :], std[pr, :])
                nc.vector.scalar_tensor_tensor(
                    shift[pr, :], mv[pr, 0:1], -1.0, rstd[pr, :],
                    op0=OP.mult, op1=OP.mult,
                )
                yn = sp.tile([128, D], F32, tag="yn", name=f"yn_{ip}")
                rows = out_d[tok0 : tok0 + tsz, :]
                for n in range(2):
                    sl = slice(n * 512, (n + 1) * 512)
                    nc.scalar.activation(
                        yn[pr, sl],
                        ps2[n][pr, :],
                        AF.Identity,
                        bias=shift[pr, :],
                        scale=rstd[pr, :],
                    )
                    if with_affine:
                        og = sp.tile([128, 512], F32, tag="og", name=f"og_{ip}")
                        nc.vector.scalar_tensor_tensor(
                            og[pr, :], yn[pr, sl], 1.0, gb_sb[pr, 0:D][:, sl],
                            op0=OP.mult, op1=OP.mult,
                        )
                        nc.vector.tensor_add(
                            og[pr, :], og[pr, :], gb_sb[pr, D : 2 * D][:, sl]
                        )
                        nc.scalar.dma_start(rows[pr, sl], og[pr, :])
                    else:
                        nc.scalar.dma_start(rows[pr, sl], yn[pr, sl])

            def stage2_mouter(gt, tiles, tok0):
                """Y = G W2 for chunk 0: m(contraction)-outer over the first
                nt-1 token tiles so w2 can stream in; the last tile runs
                t-outer afterwards, releasing most PSUM banks early."""
                nt = len(tiles)
                nm = nt - 1 if nt > 1 else nt
                toks = np.cumsum([0] + tiles).tolist()
                ps2 = [
                    [ps.tile([128, 512], F32, tag="ps", name=f"ps2_{t}_{n}") for n in range(2)]
                    for t in range(nm)
                ]
                for m in range(MH):
                    for t in range(nm):
                        lhsT = gt[m][:, toks[t] : toks[t + 1]]
                        for n in range(2):
                            nc.tensor.matmul(
                                ps2[t][n][: tiles[t], :],
                                lhsT,
                                w2_slice(m, n),
                                start=(m == 0),
                                stop=(m == MH - 1),
                            )
                for t in range(nm):
                    epilogue(ps2[t], tok0 + toks[t], tiles[t])
                for t in range(nm, nt):
                    psl = [ps.tile([128, 512], F32, tag="ps", name=f"ps2l_{t}_{n}") for n in range(2)]
                    for m in range(MH):
                        lhsT = gt[m][:, toks[t] : toks[t + 1]]
                        for n in range(2):
                            nc.tensor.matmul(
                                psl[n][: tiles[t], :],
                                lhsT,
                                w2_slice(m, n),
                                start=(m == 0),
                                stop=(m == MH - 1),
                            )
                    epilogue(psl, tok0 + toks[t], tiles[t])

            def stage2_touter(gt, tiles, tok0):
                """Y = G W2, t-outer: low PSUM pressure, w2 already resident."""
                toks = np.cumsum([0] + tiles).tolist()
                for t in range(len(tiles)):
                    ps2 = [ps.tile([128, 512], F32, tag="ps", name=f"ps2t_{t}_{n}") for n in range(2)]
                    for m in range(MH):
                        lhsT = gt[m][:, toks[t] : toks[t + 1]]
                        for n in range(2):
                            nc.tensor.matmul(
                                ps2[n][: tiles[t], :],
                                lhsT,
                                w2_slice(m, n),
                                start=(m == 0),
                                stop=(m == MH - 1),
                            )
                    epilogue(ps2, tok0 + toks[t], tiles[t])

            if len(chunks) > 1:
                xt_next = load_xt(1, offs[1], chunks[1])
            stage2_mouter(gt, tchunks[0], 0)

            for ci in range(1, len(chunks)):
                cs = chunks[ci]
                gt = stage1(xt_next, cs)
                if ci + 1 < len(chunks):
                    xt_next = load_xt(ci + 1, offs[ci + 1], chunks[ci + 1])
                stage2_touter(gt, tchunks[ci], offs[ci])

            rep_ctx.__exit__(None, None, None)

    nc.compile()
    return nc


_PROGRAM_CACHE = {}


def _get_program(C, with_affine):
    key = (C, with_affine)
    if key not in _PROGRAM_CACHE:
        _PROGRAM_CACHE[key] = build_program(C, with_affine)
    return _PROGRAM_CACHE[key]


def prepare(x, gate_w, expert_w1, expert_w2, ln_gamma, ln_beta):
    """Host-side routing + sharding. Returns (nc, in_maps, meta)."""
    x = np.asarray(x, dtype=np.float32)
    gate_w = np.asarray(gate_w, dtype=np.float32)
    expert_w1 = np.asarray(expert_w1, dtype=np.float32)
    expert_w2 = np.asarray(expert_w2, dtype=np.float32)
    ln_gamma = np.asarray(ln_gamma, dtype=np.float32)
    ln_beta = np.asarray(ln_beta, dtype=np.float32)

    B, T, _ = x.shape
    N = B * T
    xf = x.reshape(N, D)

    # Top-1 routing on host, in f64 (the f32 top-2 logit gap of this problem's
    # inputs is >8e-5, far above any rounding ambiguity).
    logits = xf.astype(np.float64) @ gate_w.astype(np.float64).T
    top = np.argmax(logits, axis=1)
    idx = [np.nonzero(top == e)[0] for e in range(E)]
    counts = [len(i) for i in idx]

    C = max(256, -(-max(counts) // 32) * 32)

    with_affine = not (np.all(ln_gamma == 1.0) and np.all(ln_beta == 0.0))
    nc = _get_program(C, with_affine)

    if with_affine:
        gb = np.empty((128, 2 * D), np.float32)
        gb[:, :D] = ln_gamma
        gb[:, D:] = ln_beta

    in_maps = []
    for e in range(E):
        npdt = np.float16 if DEFAULT_FP16 else np.float32
        xp = np.zeros((C, D), npdt)
        xp[: counts[e]] = xf[idx[e]]
        xt = np.ascontiguousarray(
            xp.T.reshape(KD, 128, C).transpose(1, 0, 2)
        )  # [128, KD, C]; xt[p,k,t] = xp[t, k*128+p]
        w1 = np.ascontiguousarray(expert_w1[e].reshape(KD, 128, H), dtype=npdt)
        w2 = np.ascontiguousarray(
            expert_w2[e]
            .reshape(KD, 2, 128, D)
            .transpose(0, 2, 1, 3)
            .reshape(KD, 128, H),
            dtype=npdt,
        )  # tile j holds m-tiles 2j | 2j+1 side by side
        m = {"xt": xt, "w1": w1, "w2": w2}
        if with_affine:
            m["gb"] = gb
        in_maps.append(m)

    return nc, in_maps, (idx, counts, B, T, N)


def assemble(results, meta):
    idx, counts, B, T, N = meta
    out = np.empty((N, D), np.float32)
    for e in range(E):
        out[idx[e]] = results[e]["out"][: counts[e]]
    return out.reshape(B, T, D)


def kernel(x, gate_w, expert_w1, expert_w2, ln_gamma, ln_beta):
    nc, in_maps, meta = prepare(x, gate_w, expert_w1, expert_w2, ln_gamma, ln_beta)
    res = run_bass_kernel_spmd(nc, in_maps, core_ids=list(range(E)))
    return assemble(res.results, meta)
